# revision 1
# baseline (speedup 1.0000x reference)
"""Trainium2 Bass kernel for nn_Demolition_splitweight_Conv2d.

Computation (per batch element b, one NeuronCore each):
    out[o, p] = (1/(127*Q)) * sum_k wvec[k] * sum_c round(Q*(conv3x3(x[c]; w[k,c,o]) + b[k,c,o]))
with Q = 12.5, wvec = [-128, 1, 2, 4, 8, 16, 32, 64].

Key trick: the per-(k,c) round-to-nearest-even happens INSIDE the TensorEngine
matmul accumulation via the fp32 magic-number trick: bf16 matmuls on TRN2
accumulate strictly row-sequentially in fp32, so a contraction-row layout of
    [27 split-product tap rows, bias_hi, bias_lo, +M, -M]   (M = 1.5*2^23)
per channel c yields exactly round(Q*y_c) added into PSUM — conv +
per-channel quantization + channel-sum is pure matmul work.

Precision: 3-term Dekker split (w_hi*x_hi + w_hi*x_lo + w_lo*x_hi), bf16
inputs / fp32 accumulation.

Layout: data-parallel over batch (8 cores). Per core the host pre-builds a
"replicated tap image" REP [128, 8*PSZA]: partition (cl*32 + t) holds the
zero-padded channel image of c = 4q + cl (q indexes the free-dim block),
pre-shifted by tap t's (dy, dx) — so every conv matmul's moving operand is a
plain contiguous AP slice and the whole input side uploads in a handful of
large DMAs.
"""

import numpy as np
import ml_dtypes

import concourse.bass as bass
import concourse.mybir as mybir
from concourse.ap import AP
from concourse.tile import TileContext
from concourse.bass_utils import run_bass_kernel_spmd

# problem dims (hardcoded per the task contract)
B, C, OUT, H, W = 8, 32, 32, 64, 64
KBITS = 8
Q = 12.5
MAGIC = 12582912.0  # 1.5 * 2^23
WVEC = np.array([-128, 1, 2, 4, 8, 16, 32, 64], np.float32)
SCALE = float(1.0 / (127.0 * Q))

PW = 66            # padded width  (1 + 64 + 1)
PH = 66            # padded height (1 + 64 + 1)
PSZ = PH * PW      # 4356
PSZA = 4232        # q-block row length: max window col 4224 (+ pad)
ROW = 8 * PSZA     # REP free width: 8 chunk blocks side by side
NROW = 7           # image rows per pixel block
NPIX = NROW * PW   # 462 = matmul moving free dim; 462*4B < 2KB PSUM bank
NPB = 10           # pixel blocks (rows 0..62 in blocks of 7; block 9 covers row 63)

# upload segments (REP free-dim ranges, aligned to pixel blocks)
SEGS = [(i * NPIX, (i + 1) * NPIX) for i in range(9)] + [(9 * NPIX, PSZA)]
# per-block moving free dim: 9 blocks of 7 rows + 1 block covering row 63 only
NPIX_PB = [NPIX] * 9 + [PW]

BF = mybir.dt.bfloat16
F32 = mybir.dt.float32

_cache = {}


def _bf16(a):
    return np.asarray(a, np.float32).astype(ml_dtypes.bfloat16)


def _prep_weights(weight, bias):
    """Build lhsT [128, 16*128] and gmat [128, 64] host arrays (bf16)."""
    qw = (Q * weight.astype(np.float32)).reshape(KBITS, C, OUT, 9)  # [k,c,o,t]
    whi = _bf16(qw).astype(np.float32)
    wlo = _bf16(qw - whi).astype(np.float32)
    qb = (Q * bias.astype(np.float32)).reshape(KBITS, C, OUT)
    bhi = _bf16(qb).astype(np.float32)
    blo = _bf16(qb - bhi).astype(np.float32)

    # mat[p, j]: p = cl*32 + t (c = 4q + cl), j = k*16 + ol, o = mblk*16 + ol
    k_of = np.arange(128) // 16
    ol_of = np.arange(128) % 16
    lhsT = np.zeros((128, 16, 128), np.float32)
    for mblk in range(2):
        o_of = mblk * 16 + ol_of
        for q in range(8):
            mat = np.zeros((128, 128), np.float32)
            for cl in range(4):
                c = q * 4 + cl
                r0 = cl * 32
                mat[r0 + 0 : r0 + 9, :] = whi[k_of, c, o_of].T
                mat[r0 + 9 : r0 + 18, :] = whi[k_of, c, o_of].T
                mat[r0 + 18 : r0 + 27, :] = wlo[k_of, c, o_of].T
                mat[r0 + 27, :] = bhi[k_of, c, o_of]
                mat[r0 + 28, :] = blo[k_of, c, o_of]
                mat[r0 + 29, :] = MAGIC
                mat[r0 + 30, :] = -MAGIC
                # r0+31 stays 0 (pad row; rhs content is ones)
            lhsT[:, mblk * 8 + q, :] = mat
    lhsT_bf = _bf16(lhsT.reshape(128, 16 * 128))

    gmat = np.zeros((128, 64), np.float32)
    j = np.arange(128)
    gmat[j, ol_of] = WVEC[k_of]             # mblk0 -> out cols 0..15
    gmat[j, 32 + 16 + ol_of] = WVEC[k_of]   # mblk1 -> out cols 16..31
    return lhsT_bf, _bf16(gmat)


def _build_xrep(x):
    """Host-side REP construction: [B, 128, 8*PSZA] bf16.

    partition p = cl*32 + t, free block q; channel c = 4q + cl:
      t 0..8   : padded bf16(x_hi)[c] shifted by (dy, dx) = (t//3, t%3)
      t 9..17  : padded x_lo shifted
      t 18..26 : padded x_hi shifted (dup for the w_lo rows)
      t 27..31 : ones
    """
    xh = _bf16(x)
    xl = _bf16(x - xh.astype(np.float32))

    PADW = PSZA + 140
    def pad(img):  # [B, C, H, W] bf16 -> [B, C, PADW]
        p = np.zeros((B, C, PADW), ml_dtypes.bfloat16)
        p[:, :, :PSZ].reshape(B, C, PH, PW)[:, :, 1 : H + 1, 1 : W + 1] = img
        return p

    ph, pl = pad(xh), pad(xl)
    offs = [(t // 3) * PW + (t % 3) for t in range(9)]
    # shifted[s, b, c, j] = pad[b, c, offs[s] + j]
    sh_h = np.stack([ph[:, :, o : o + PSZA] for o in offs], axis=0)
    sh_l = np.stack([pl[:, :, o : o + PSZA] for o in offs], axis=0)

    xrep = np.zeros((B, 4, 32, 8, PSZA), ml_dtypes.bfloat16)
    cidx = (4 * np.arange(8)[None, :] + np.arange(4)[:, None])  # [cl, q] -> c
    # [9, B, cl, q, PSZ] views
    hi = sh_h[:, :, cidx, :].transpose(1, 2, 0, 3, 4)  # [B, cl, 9, q, PSZ]
    lo = sh_l[:, :, cidx, :].transpose(1, 2, 0, 3, 4)
    xrep[:, :, 0:9, :, :] = hi
    xrep[:, :, 9:18, :, :] = lo
    xrep[:, :, 18:27, :, :] = hi
    xrep[:, :, 27:32, :, :] = ml_dtypes.bfloat16(1.0)
    return xrep.reshape(B, 128, ROW)


def _split_multiwaits(nc):
    """This container's walrus allows one sync-wait per instruction; move
    extras onto preceding same-engine NoOps."""
    for bb in nc.main_func.blocks:
        insts = bb.instructions
        i = 0
        while i < len(insts):
            ins = insts[i]
            si = getattr(ins, "sync_info", None)
            if si is not None and si.on_wait is not None and len(si.on_wait) > 1:
                waits = list(si.on_wait)
                nops = []
                for j, w in enumerate(waits[:-1]):
                    nop = mybir.InstNoOp(name=f"{ins.name}-wsplit{j}", ins=[], outs=[])
                    nop.engine = ins.engine
                    nop.sync_info = mybir.SyncInfo(on_wait=[w], on_update=[])
                    nops.append(nop)
                si.on_wait = [waits[-1]]
                ins.sync_info = si
                for j, nop in enumerate(nops):
                    insts.insert(i + j, nop)
                i += len(nops)
            i += 1


def _build_nc():
    nc = bass.Bass()
    xrep_d = nc.dram_tensor("xrep", [128, ROW], BF, kind="ExternalInput")
    wc_d = nc.dram_tensor("wconst", [128, 16 * 128 + 64], BF, kind="ExternalInput")
    out_d = nc.dram_tensor("out", [OUT, H * W], F32, kind="ExternalOutput")

    with TileContext(nc) as tc:
        with (
            tc.tile_pool(name="const", bufs=1) as cpool,
            tc.tile_pool(name="work", bufs=6) as wpool,
            tc.tile_pool(name="outp", bufs=4) as opool,
            tc.tile_pool(name="psP", bufs=5, space="PSUM") as psP,
            tc.tile_pool(name="psR", bufs=2, space="PSUM") as psR,
        ):
            wconst = cpool.tile([128, 16 * 128 + 64], BF, tag="wconst")
            lhsT = wconst[:, 0 : 16 * 128]
            gmat = wconst[:, 16 * 128 : 16 * 128 + 64]
            nc.sync.dma_start(out=wconst[:, 0:256], in_=wc_d[:, 0:256])

            rep = cpool.tile([128, ROW], BF, tag="rep")

            def seg_dma(s0, s1):
                dst = AP(tensor=rep.tensor, offset=rep.offset + s0,
                         ap=[[ROW, 128], [PSZA, 8], [1, s1 - s0]])
                src = AP(tensor=xrep_d, offset=s0,
                         ap=[[ROW, 128], [PSZA, 8], [1, s1 - s0]])
                nc.sync.dma_start(out=dst, in_=src)

            act_warm = opool.tile([32, NPIX], F32, tag="osb", name="actwarm")
            nc.scalar.copy(act_warm[:, 0:64], gmat[0:32, :])
            warm_ps = psP.tile([128, NPIX], F32, tag="P", name="warmps")
            for wi in range(12):
                nc.tensor.matmul(warm_ps[:, :256], lhsT[:, 0:128],
                                 lhsT[:, 0:256], start=(wi == 0),
                                 stop=(wi == 11))
            seg_dma(*SEGS[0])
            nc.sync.dma_start(out=wconst[:, 256:], in_=wc_d[:, 256:])
            for s0, s1 in SEGS[1:]:
                seg_dma(s0, s1)

            for pb in range(NPB):
                base = pb * NPIX
                n = NPIX_PB[pb]
                a_tiles = []
                P01 = [psP.tile([128, NPIX], F32, tag="P", name=f"P{pb}_{i}") for i in range(2)]
                for q in range(8):
                    rhs = rep[:, q * PSZA + base : q * PSZA + base + n]
                    for m in range(2):
                        w_ap = lhsT[:, (m * 8 + q) * 128 : (m * 8 + q + 1) * 128]
                        nc.tensor.matmul(P01[m][:, :n], w_ap, rhs,
                                         start=(q == 0), stop=(q == 7))
                for m in range(2):
                    A = wpool.tile([128, NPIX], BF, tag="A")
                    if m == 0:
                        nc.vector.tensor_copy(A[:, :n], P01[m][:, :n])
                    else:
                        nc.scalar.copy(A[:, :n], P01[m][:, :n])
                    a_tiles.append(A)
                R = psR.tile([32, NPIX], F32, tag="R")
                nc.tensor.matmul(R[:, :n], gmat[:, 0:32], a_tiles[0][:, :n],
                                 start=True, stop=False)
                nc.tensor.matmul(R[:, :n], gmat[:, 32:64], a_tiles[1][:, :n],
                                 start=False, stop=True)
                osb = opool.tile([32, NPIX], F32, tag="osb")
                nc.scalar.mul(osb[:, :n], R[:, :n], SCALE)

                nr = NROW if pb < NPB - 1 else H - (NPB - 1) * NROW
                dst = AP(tensor=out_d, offset=pb * NROW * W,
                         ap=[[H * W, OUT], [W, nr], [1, W]])
                s = AP(tensor=osb.tensor, offset=osb.offset,
                       ap=[[NPIX, 32], [PW, nr], [1, W]])
                nc.sync.dma_start(out=dst, in_=s)

    _split_multiwaits(nc)
    return nc


def kernel(x, weight, bias):
    x = np.asarray(x, np.float32)
    weight = np.asarray(weight, np.float32)
    bias = np.asarray(bias, np.float32)

    xrep = _build_xrep(x)
    lhsT, gmat = _prep_weights(weight, bias)
    wconst = np.concatenate([lhsT, gmat], axis=1)

    if "nc" not in _cache:
        _cache["nc"] = _build_nc()
    nc = _cache["nc"]

    in_maps = [{"xrep": xrep[b], "wconst": wconst} for b in range(B)]
    res = run_bass_kernel_spmd(nc, in_maps, core_ids=list(range(B)))
    out = np.stack([r["out"] for r in res.results])
    return out.reshape(B, OUT, H, W).astype(np.float32)



# revision 6
# speedup vs baseline: 1.4722x; 1.4722x over previous
"""Trainium2 Bass kernel for nn_Demolition_splitweight_Conv2d.

Computation (per batch element b, one NeuronCore each):
    out[o, p] = (1/(127*Q)) * sum_k wvec[k] * sum_c round(Q*(conv3x3(x[c]; w[k,c,o]) + b[k,c,o]))
with Q = 12.5, wvec = [-128, 1, 2, 4, 8, 16, 32, 64].

The per-(k,c) round-to-nearest happens INSIDE the TensorEngine matmul
accumulation via the fp32 magic-number trick. The PE accumulates partial
sums sequentially within 16-row contraction sections (and combines
sections exactly in fp32-integer range), so each channel gets a 16-row
band aligned to a section:

    rows 0-8 : fp16 x taps (pre-shifted padded image rows)
    row  9   : bias_hi/1024   (rhs row holds 1024.0)
    row 10   : bias_lo/1024   (rhs 1024.0)
    row 11   : +12288         (rhs 1024.0 -> product +1.5*2^23 = magic)
    row 12   : -12288         (rhs 1024.0)
    rows 13-15: zero weights  (rhs 1024.0)

fp16 (10-bit mantissa) makes the Dekker splits of the bf16 design
unnecessary: one product row per tap. 32 bands = 512 rows = 4 chained
128-row matmuls per output half; + 2 gmat matmuls for the wvec k-sum
=> 10 matmuls per pixel block (vs 18 for the bf16 3-split layout).

Layout: data-parallel over batch (8 cores). REP [128, 4*PSZA] fp16:
partition p = 16*band + row, chunk cc (free blocks of PSZA) holds
channels ch = 8*cc + band. Only the 9 tap rows per band are uploaded
from HBM (2.4 MB); the constant-1024 rows are memset on GpSimd per
column window, pipelined ahead of the PE.
"""

import numpy as np
import ml_dtypes

import concourse.bass as bass
import concourse.mybir as mybir
from concourse.ap import AP
from concourse.tile import TileContext
from concourse.bass_utils import run_bass_kernel_spmd

# problem dims (hardcoded per the task contract)
B, C, OUT, H, W = 8, 32, 32, 64, 64
KBITS = 8
Q = 12.5
WVEC = np.array([-128, 1, 2, 4, 8, 16, 32, 64], np.float32)
SCALE = float(1.0 / (127.0 * Q))
MAG_LHS = 12288.0          # * 1024 (rhs) = 1.5*2^23
ONESV = 1024.0

PW = 66                    # padded width  (1 + 64 + 1)
PH = 66                    # padded height
PSZ = PH * PW              # 4356
USED = 4224                # max window col (64 rows of 66) (+ row 63 window)
PSZA = 4232                # chunk pitch in REP free dim
NCH = 4                    # chunks; chunk cc holds channels 8*cc..8*cc+7
NROW = 7                   # image rows per pixel block
NPIX = NROW * PW           # 462 = matmul moving free dim
NPB = 10                   # pixel blocks: 9 of 7 rows + 1 of 1 row
NPIX_PB = [NPIX] * 9 + [PW]

# upload/memset column windows: block0 alone first (fast start), then 2-block
WINS = [(0, 462), (462, 924), (924, 1848), (1848, 2772), (2772, 3696), (3696, 4224)]
# first block covered by each window (ceil(start/NPIX))
WIN_FIRST_BLOCK = [0, 1, 2, 4, 6, 8]

F16 = mybir.dt.float16
BF = mybir.dt.bfloat16
F32 = mybir.dt.float32

_cache = {}


def _prep_weights(weight, bias):
    """wconst16 [128, 8*128] fp16 and gmat [128, 64] bf16 host arrays."""
    w16 = np.asarray(Q * weight.astype(np.float32), np.float32).reshape(KBITS, C, OUT, 9)
    w16 = w16.astype(np.float16)
    qb = (Q * bias.astype(np.float32)).reshape(KBITS, C, OUT)
    bh = (qb / ONESV).astype(np.float16)
    bl = ((qb - bh.astype(np.float32) * ONESV) / ONESV).astype(np.float16)

    k_of = np.arange(128) // 16          # lhsT column j = k*16 + ol
    ol_of = np.arange(128) % 16
    wc = np.zeros((128, 8, 128), np.float16)   # [partition, m*4+cc, col]
    for m in range(2):
        o_of = m * 16 + ol_of
        for cc in range(NCH):
            mat = np.zeros((128, 128), np.float16)
            for band in range(8):
                ch = 8 * cc + band
                r0 = 16 * band
                mat[r0 : r0 + 9, :] = w16[k_of, ch, o_of].T
                mat[r0 + 9, :] = bh[k_of, ch, o_of]
                mat[r0 + 10, :] = bl[k_of, ch, o_of]
                mat[r0 + 11, :] = MAG_LHS
                mat[r0 + 12, :] = -MAG_LHS
            wc[:, m * 4 + cc, :] = mat
    wc = wc.reshape(128, 8 * 128)

    gmat = np.zeros((128, 64), np.float32)
    j = np.arange(128)
    gmat[j, ol_of] = WVEC[k_of]              # m0 -> out cols 0..15
    gmat[j, 32 + 16 + ol_of] = WVEC[k_of]    # m1 -> out cols 16..31
    return wc, gmat.astype(ml_dtypes.bfloat16)


def _build_xrep(x):
    """Host REP: [B, 8 band, 16 row, 4 chunk, USED] fp16.

    band b, chunk cc -> channel ch = 8*cc + b; rows 0-8 are tap-shifted
    padded images (tap t shift (t//3, t%3)), rows 9-15 hold 1024.0 for
    the bias/magic products.
    """
    x16 = np.asarray(x, np.float32).astype(np.float16)
    padw = PSZ + PW * 2 + 8
    p = np.zeros((B, C, padw), np.float16)
    p[:, :, :PSZ].reshape(B, C, PH, PW)[:, :, 1 : H + 1, 1 : W + 1] = x16
    offs = [(t // 3) * PW + (t % 3) for t in range(9)]
    sh = np.stack([p[:, :, o : o + USED] for o in offs], axis=2)  # [B, C, 9, USED]
    xrep = np.full((B, 8, 16, NCH, USED), np.float16(ONESV), np.float16)
    # taps: [B, cc, band, 9, USED] -> [B, band, 9, cc, USED]
    xrep[:, :, 0:9, :, :] = sh.reshape(B, NCH, 8, 9, USED).transpose(0, 2, 3, 1, 4)
    return np.ascontiguousarray(xrep)


def _split_multiwaits(nc):
    """This container's walrus allows one sync-wait per instruction; move
    extras onto preceding same-engine NoOps."""
    for bb in nc.main_func.blocks:
        insts = bb.instructions
        i = 0
        while i < len(insts):
            ins = insts[i]
            si = getattr(ins, "sync_info", None)
            if si is not None and si.on_wait is not None and len(si.on_wait) > 1:
                waits = list(si.on_wait)
                nops = []
                for j, w in enumerate(waits[:-1]):
                    nop = mybir.InstNoOp(name=f"{ins.name}-wsplit{j}", ins=[], outs=[])
                    nop.engine = ins.engine
                    nop.sync_info = mybir.SyncInfo(on_wait=[w], on_update=[])
                    nops.append(nop)
                si.on_wait = [waits[-1]]
                ins.sync_info = si
                for j, nop in enumerate(nops):
                    insts.insert(i + j, nop)
                i += len(nops)
            i += 1


def _build_nc():
    nc = bass.Bass()
    xrep_d = nc.dram_tensor("xrep", [128 * NCH, USED], F16, kind="ExternalInput")
    wc_d = nc.dram_tensor("wc16", [128, 8 * 128], F16, kind="ExternalInput")
    gm_d = nc.dram_tensor("gmat", [128, 64], BF, kind="ExternalInput")
    out_d = nc.dram_tensor("out", [OUT, H * W], F32, kind="ExternalOutput")

    with TileContext(nc) as tc:
        with (
            tc.tile_pool(name="const", bufs=1) as cpool,
            tc.tile_pool(name="work", bufs=6) as wpool,
            tc.tile_pool(name="outp", bufs=4) as opool,
            tc.tile_pool(name="psP", bufs=5, space="PSUM") as psP,
            tc.tile_pool(name="psR", bufs=2, space="PSUM") as psR,
        ):
            wc16 = cpool.tile([128, 8 * 128], F16, tag="wc16")
            gmat = cpool.tile([128, 64], BF, tag="gmat")
            rep = cpool.tile([128, NCH * PSZA], F16, tag="rep")
            RPITCH = NCH * PSZA

            # m0 lhsT first: the first conv matmuls need only cols 0..511
            nc.sync.dma_start(out=wc16[:, 0:512], in_=wc_d[:, 0:512])

            def win_dma(s0, s1):
                dst = AP(tensor=rep.tensor, offset=rep.offset + s0,
                         ap=[[RPITCH, 128], [PSZA, NCH], [1, s1 - s0]])
                src = AP(tensor=xrep_d, offset=s0,
                         ap=[[NCH * USED, 128], [USED, NCH], [1, s1 - s0]])
                nc.sync.dma_start(out=dst, in_=src)

            win_dma(*WINS[0])
            nc.sync.dma_start(out=wc16[:, 512:1024], in_=wc_d[:, 512:1024])
            nc.sync.dma_start(out=gmat[:, :], in_=gm_d[:, :])
            for s0, s1 in WINS[1:]:
                win_dma(s0, s1)

            for pb in range(NPB):
                base = pb * NPIX
                n = NPIX_PB[pb]
                P01 = [psP.tile([128, NPIX], F32, tag="P", name=f"P{pb}_{i}")
                       for i in range(2)]
                for cc in range(NCH):
                    rhs = rep[:, cc * PSZA + base : cc * PSZA + base + n]
                    for m in range(2):
                        w_ap = wc16[:, (m * 4 + cc) * 128 : (m * 4 + cc + 1) * 128]
                        nc.tensor.matmul(P01[m][:, :n], w_ap, rhs,
                                         start=(cc == 0), stop=(cc == NCH - 1))
                a_tiles = []
                for m in range(2):
                    A = wpool.tile([128, NPIX], BF, tag="A")
                    if m == 0:
                        nc.vector.tensor_copy(A[:, :n], P01[m][:, :n])
                    else:
                        nc.scalar.copy(A[:, :n], P01[m][:, :n])
                    a_tiles.append(A)
                R = psR.tile([32, NPIX], F32, tag="R")
                nc.tensor.matmul(R[:, :n], gmat[:, 0:32], a_tiles[0][:, :n],
                                 start=True, stop=False)
                nc.tensor.matmul(R[:, :n], gmat[:, 32:64], a_tiles[1][:, :n],
                                 start=False, stop=True)

                nr = NROW if pb < NPB - 1 else H - (NPB - 1) * NROW
                osb = opool.tile([32, NROW * W], F32, tag="osb")
                rsrc = AP(tensor=R.tensor, offset=R.offset,
                          ap=[[NPIX, 32], [PW, nr], [1, W]])
                nc.scalar.mul(osb[:, : nr * W], rsrc, SCALE)
                dst = AP(tensor=out_d, offset=pb * NROW * W,
                         ap=[[H * W, OUT], [1, nr * W]])
                s = AP(tensor=osb.tensor, offset=osb.offset,
                       ap=[[NROW * W, 32], [1, nr * W]])
                nc.scalar.dma_start(out=dst, in_=s)

    _split_multiwaits(nc)
    return nc


def kernel(x, weight, bias):
    x = np.asarray(x, np.float32)
    weight = np.asarray(weight, np.float32)
    bias = np.asarray(bias, np.float32)

    xrep = _build_xrep(x).reshape(B, 128 * NCH, USED)
    wc16, gmat = _prep_weights(weight, bias)

    if "nc" not in _cache:
        _cache["nc"] = _build_nc()
    nc = _cache["nc"]

    in_maps = [{"xrep": xrep[b], "wc16": wc16, "gmat": gmat} for b in range(B)]
    res = run_bass_kernel_spmd(nc, in_maps, core_ids=list(range(B)))
    out = np.stack([r["out"] for r in res.results])
    return out.reshape(B, OUT, H, W).astype(np.float32)
